# revision 32
# baseline (speedup 1.0000x reference)
"""Trainium2 Bass kernel for nn_CoreRNNFW (fast-weight RNN with inner recall loop).

Strategy (v4):
- Pure data parallel over batch B=256 -> 32 samples per core on 8 cores.
- Rank-t factorization of the Hebbian fast-weight matrix A_t:
    A_t = sum_tau eta*lam^(t-1-tau) u_tau u_tau^T
  A is never materialized. Coefficients are folded into storage:
    Ujb (U in [j, q] layout) is scaled by g(tau) = eta*lam^(-1-tau) at append;
    the remaining factor f(t) = lam^t rides in the mask (maskf).
- All matmul operands bf16 (4x PE throughput vs fp32), fp32 PSUM accumulate.
- z @ Wg^T + b_h precomputed for all T in one batched GEMM at kernel start;
  injected per-step via selector matmul.
- LayerNorm+ReLU with rs deferred off the critical path:
    relu(LN(x)) = rs * relu(x - mu)   (rs = 1/sqrt(var+eps) > 0)
  The chain runs bn_stats -> aggr -> (-mu) -> Relu(x + bias=-mu) only.
  rs is computed in parallel (sqrt+recip) and folded in later:
    * for the recall matmul G: w = (maskf*rs4) (.) G  (rs4 = rs replicated
      across the 4 partition groups, built by a tiny mask^T matmul)
    * for U appends: append-TS scales by rs4 (and g(tau) for Ujb)
    * for h_base/head consumers: one cheap TS builds true hT = rs4*hT'.
- PSUM x is built in two i-halves (separate tiles) so bn_stats(half0)
  overlaps the half-1 matmuls.
"""
import sys

sys.path.insert(0, "/opt/trn_rl_repo")

import numpy as np
import ml_dtypes
import concourse.bass as bass
import concourse.bacc as bacc
import concourse.tile as tile
from concourse import mybir
from concourse.bass_utils import run_bass_kernel_spmd

BF16 = ml_dtypes.bfloat16
T, B, D_G, D_H, D_OUT = 24, 256, 256, 512, 256
S_INNER = 3
LAM, ETA = 0.95, 0.5
LN_EPS = 1e-5
N_CORES = 8
BC = B // N_CORES            # 32 samples per core
NQ = T * BC                  # 768 q-slots (tau-major: q = tau*32 + b)
NKC = NQ // 128              # 6 q-chunks of 128
HALF = D_H // 2
F32 = mybir.dt.float32
BF = mybir.dt.bfloat16
F8 = mybir.dt.float8e4
PM_DR = mybir.MatmulPerfMode.DoubleRow
AL = mybir.AluOpType
AF = mybir.ActivationFunctionType


def _build_nc(fast_ln=True):
    nc = bacc.Bacc(None, target_bir_lowering=False, debug=False)

    zTa = nc.dram_tensor("zTa", [128, 3, NQ], BF, kind="ExternalInput")
    WgTa = nc.dram_tensor("WgTa", [128, 3, 2 * (HALF + 1)], BF, kind="ExternalInput")
    WhT = nc.dram_tensor("WhT", [128, 4, 2 * (HALF + 1)], BF, kind="ExternalInput")
    HWT = nc.dram_tensor("HWT", [128, 4, D_OUT], BF, kind="ExternalInput")
    sel4 = nc.dram_tensor("sel4", [128, 4, BC], BF, kind="ExternalInput")
    id32 = nc.dram_tensor("id32", [BC, BC], BF, kind="ExternalInput")
    mask16 = nc.dram_tensor("mask16", [128, BC], BF, kind="ExternalInput")
    maskT = nc.dram_tensor("maskT", [BC, 128], F32, kind="ExternalInput")
    cleanv = nc.dram_tensor("cleanv", [BC, D_OUT], F32, kind="ExternalInput")
    hbias_r = nc.dram_tensor("hbias_r", [BC, D_OUT], F32, kind="ExternalInput")
    if not fast_ln:
        gam_r = nc.dram_tensor("gam_r", [BC, D_H], F32, kind="ExternalInput")
        bet_r = nc.dram_tensor("bet_r", [BC, D_H], F32, kind="ExternalInput")

    partial = nc.dram_tensor("partial", [BC], F32, kind="ExternalOutput")
    h_final = nc.dram_tensor("h_final", [BC, D_H], F32, kind="ExternalOutput")

    with tile.TileContext(nc) as tc:
        with (
            tc.tile_pool(name="persist", bufs=1) as P,
            tc.tile_pool(name="work", bufs=3) as W,
            tc.tile_pool(name="stats", bufs=8) as ST,
            tc.tile_pool(name="psA", bufs=2, space="PSUM") as PSA,
            tc.tile_pool(name="psG", bufs=2, space="PSUM") as PSG,
            tc.tile_pool(name="psT", bufs=1, space="PSUM") as PST,
            tc.tile_pool(name="psS", bufs=1, space="PSUM") as PSS,
        ):
            # ---- persistent SBUF ----
            zTa_sb = P.tile([128, 3, NQ], BF)
            WgTa_sb = P.tile([128, 3, 2 * (HALF + 1)], BF)
            WhT_sb = P.tile([128, 4, 2 * (HALF + 1)], BF)
            HWT_sb = P.tile([128, 4, D_OUT], BF)
            sel4_sb = P.tile([128, 4, BC], BF)
            id_sb = P.tile([BC, BC], BF)
            mask_sb = P.tile([128, BC], BF)
            maskT_sb = P.tile([BC, 128], F32)
            cv_sb = P.tile([BC, D_OUT], F32)
            hbias_sb = P.tile([BC, D_OUT], F32)
            eps_sb = P.tile([BC, 1], F32)
            if not fast_ln:
                gam_sb = P.tile([BC, D_H], F32)
                bet_sb = P.tile([BC, D_H], F32)

            zW_sbs = [P.tile([128, 2 * (HALF + 1)], BF, name=f"zW{i}")
                      for i in range(NKC)]  # z@Wg^T+b_h +augs, per group
            Ujb = P.tile([128, 4, NQ], BF)        # g(tau)*rs-scaled U, [j, q]
            Upi = P.tile([128, NKC, 2, HALF + 1], F8)  # 4rs*U +aug, [q,i] fp8
            ones_sb = P.tile([128, 1], BF)        # for column sums
            maskf = P.tile([128, BC], BF)         # mask * lam^t
            maskfrs = P.tile([128, BC], BF)       # maskf * rs4 (per stage)
            hT = P.tile([128, 4, BC], BF)         # h' (unscaled), [j, b]
            hTs = P.tile([128, 4, BC], BF)        # true h = rs*h', [j, b]
            h_sb = P.tile([BC, D_H], BF)          # h' (unscaled), [b, i]
            hb16 = P.tile([BC, 2 * (HALF + 1)], BF)  # h_base halves+aug bf16

            # ---- input DMAs ----
            nc.sync.dma_start(out=zTa_sb, in_=zTa[:])
            nc.sync.dma_start(out=WgTa_sb, in_=WgTa[:])
            nc.sync.dma_start(out=WhT_sb, in_=WhT[:])
            nc.sync.dma_start(out=HWT_sb, in_=HWT[:])
            nc.sync.dma_start(out=sel4_sb, in_=sel4[:])
            nc.sync.dma_start(out=id_sb, in_=id32[:])
            nc.sync.dma_start(out=mask_sb, in_=mask16[:])
            nc.sync.dma_start(out=maskT_sb, in_=maskT[:])
            nc.sync.dma_start(out=cv_sb, in_=cleanv[:])
            nc.sync.dma_start(out=hbias_sb, in_=hbias_r[:])
            if not fast_ln:
                nc.sync.dma_start(out=gam_sb, in_=gam_r[:])
                nc.sync.dma_start(out=bet_sb, in_=bet_r[:])
            nc.vector.memset(eps_sb, LN_EPS)
            nc.gpsimd.memset(Ujb, 0.0)
            nc.gpsimd.memset(Upi, 0.0)
            nc.vector.memset(ones_sb, 1.0)

            # ---- zW precompute: zW[(t,b), i] = z[t,b,:]@Wg^T + b_h ----
            H1 = HALF + 1
            for mc in range(NKC):
                for hz in range(2):
                    slz = slice(hz * H1, (hz + 1) * H1)
                    ps_z = PSG.tile([128, H1], F32, tag="psG")
                    for kc in range(3):
                        nc.tensor.matmul(
                            ps_z, zTa_sb[:, kc, mc * 128:(mc + 1) * 128],
                            WgTa_sb[:, kc, slz], start=(kc == 0),
                            stop=(kc == 2))
                    nc.vector.tensor_copy(zW_sbs[mc][:, slz], ps_z)

            def ln_relu(psA, psB, stats, last):
                """x split in two PSUM half-tiles [BC, HALF+1] whose col
                HALF holds sum(x-half)/32 (from aug weight columns).
                mu is formed from the aug sums (fast, on-chain); variance
                via bn_stats runs off the critical path (bn_A was emitted
                with the producing matmuls; bn_B is emitted late here).
                h_sb/hT = relu(x - mu); rs4 [128,1] PSUM built off-chain;
                maskfrs refreshed; if `last`, build true hTs instead."""
                mv = ST.tile([BC, 2], F32, tag="mv")
                nmr = ST.tile([BC, 1], F32, tag="nmr")
                smu = ST.tile([BC, 1], F32, tag="smu")
                rs = ST.tile([BC, 1], F32, tag="rs")
                # mu = (sumA + sumB)*32/512; nmr = -mu (one PSUM input per
                # instruction: scale sumA first, then fold sumB)
                nc.vector.tensor_scalar_mul(smu, psA[:, HALF:HALF + 1],
                                            -1.0 / 16.0)
                nc.vector.tensor_scalar(
                    out=nmr, in0=psB[:, HALF:HALF + 1], scalar1=-1.0 / 16.0,
                    scalar2=smu, op0=AL.mult, op1=AL.add)
                # critical path: relu with bias=-mu only; half A on ACT,
                # half B on DVE (2-op tensor_scalar add+max) in parallel
                if fast_ln:
                    nc.scalar.activation(h_sb[:, :HALF], psA[:, :HALF],
                                         AF.Relu, bias=nmr)
                    nc.vector.tensor_scalar(
                        out=h_sb[:, HALF:], in0=psB[:, :HALF], scalar1=nmr,
                        scalar2=0.0, op0=AL.add, op1=AL.max)
                else:
                    # general path: full affine LN then relu (h' still
                    # unscaled by rs; gamma/beta folded differently is not
                    # supported here, so apply full form with rs=1 trick:
                    # h' = relu((x-mu)*gamma + beta*sd)/ ... fallback to
                    # exact: h = relu((x-mu)*rs*gamma+beta), then h' = h,
                    # rs4 forced to 1 by computing rs into the apply.
                    y = W.tile([BC, D_H], F32, tag="ln_y")
                    nc.vector.bn_stats(out=stats[:, 1, :], in_=psB[:, :HALF])
                    nc.vector.bn_aggr(out=mv, in_=stats)
                    nc.scalar.activation(mv[:, 1:2], mv[:, 1:2], AF.Sqrt,
                                         bias=eps_sb)
                    nc.vector.reciprocal(rs, mv[:, 1:2])
                    nc.vector.tensor_scalar(
                        out=nmr, in0=mv[:, 0:1], scalar1=rs, scalar2=-1.0,
                        op0=AL.mult, op1=AL.mult)
                    nc.vector.tensor_scalar(
                        out=y[:, :HALF], in0=psA[:, :HALF], scalar1=rs,
                        scalar2=nmr, op0=AL.mult, op1=AL.add)
                    nc.vector.tensor_scalar(
                        out=y[:, HALF:], in0=psB[:, :HALF], scalar1=rs,
                        scalar2=nmr, op0=AL.mult, op1=AL.add)
                    nc.gpsimd.tensor_mul(y, y, gam_sb)
                    nc.vector.tensor_add(y, y, bet_sb)
                    nc.scalar.activation(h_sb, y, AF.Relu)
                    nc.vector.memset(rs, 1.0)
                psT = PST.tile([128, 4, BC], BF, tag="psT")
                for jc in range(4):
                    nc.tensor.transpose(
                        psT[:, jc, :], h_sb[:, jc * 128:(jc + 1) * 128], id_sb)
                nc.vector.tensor_copy(hT, psT)
                # off-chain: bn_B (late), var -> rs -> rs4 -> maskfrs
                ps_r4 = PSS.tile([128, 3], F32, tag="psS")
                if fast_ln:
                    nc.vector.bn_stats(out=stats[:, 1, :], in_=psB[:, :HALF])
                    nc.vector.bn_aggr(out=mv, in_=stats)
                    nc.scalar.activation(mv[:, 1:2], mv[:, 1:2], AF.Sqrt,
                                         bias=eps_sb)
                    nc.vector.reciprocal(rs, mv[:, 1:2])
                    nc.tensor.matmul(ps_r4[:, 0:1], maskT_sb, rs, start=True,
                                     stop=True)
                else:
                    one = ST.tile([BC, 1], F32, tag="one")
                    nc.vector.memset(one, 1.0)
                    nc.tensor.matmul(ps_r4[:, 0:1], maskT_sb, one, start=True,
                                     stop=True)
                if last:
                    nc.vector.tensor_scalar(
                        out=hTs, in0=hT, scalar1=ps_r4[:, 0:1], scalar2=None,
                        op0=AL.mult)
                else:
                    nc.vector.tensor_scalar(
                        out=maskfrs, in0=maskf, scalar1=ps_r4[:, 0:1],
                        scalar2=0.25, op0=AL.mult, op1=AL.mult)
                return ps_r4, rs

            # ---- main time loop ----
            for t in range(T):
                kt = (BC * t + 127) // 128   # q-chunks in use

                if t > 0:
                    nc.vector.tensor_scalar_mul(maskf, mask_sb, LAM ** t)
                else:
                    nc.vector.tensor_copy(maskf, mask_sb)

                # h_base = h@Wh^T + (z_t@Wg^T + b_h) per i-half
                psA = PSA.tile([BC, H1], F32, tag="psA")
                psB = PSA.tile([BC, H1], F32, tag="psB")
                hb_stats = ST.tile([BC, 2, 6], F32, tag="stats")
                for hf, ps in ((0, psA), (1, psB)):
                    sl = slice(hf * H1, (hf + 1) * H1)
                    n = 0
                    if t > 0:
                        for jc in range(4):
                            nc.tensor.matmul(
                                ps, hTs[:, jc, :], WhT_sb[:, jc, sl],
                                start=(n == 0), stop=False)
                            n += 1
                    nc.tensor.matmul(ps, sel4_sb[:, t % 4, :],
                                     zW_sbs[t // 4][:, sl], start=(n == 0),
                                     stop=True)
                    if hf == 0:
                        nc.vector.bn_stats(out=hb_stats[:, 0, :],
                                           in_=ps[:, :HALF])
                ps_r4, rs_sb = ln_relu(psA, psB, hb_stats, last=(t == 0))

                if t > 0:
                    nc.scalar.copy(hb16[:, :H1], psA)
                    nc.scalar.copy(hb16[:, H1:], psB)
                    for _s in range(S_INNER):
                        # G[q,b] = sum_j gU[j,q] h'[j,b]
                        ps_G = PSG.tile([128, HALF], F32, tag="psG")
                        for k in range(kt):
                            for jc in range(4):
                                nc.tensor.matmul(
                                    ps_G[:, k * BC:(k + 1) * BC],
                                    Ujb[:, jc, k * 128:(k + 1) * 128],
                                    hT[:, jc, :],
                                    start=(jc == 0), stop=(jc == 3))
                        # w = (maskf*rs4) (.) G
                        w = W.tile([128, NKC, BC], F8, tag="w")
                        mf_bc = bass.AP(
                            tensor=maskfrs.tensor, offset=maskfrs.offset,
                            ap=[maskfrs.ap[0], [0, kt], maskfrs.ap[1]])
                        g_bc = bass.AP(
                            tensor=ps_G.tensor, offset=ps_G.offset,
                            ap=[ps_G.ap[0], [BC, kt], [1, BC]])
                        nc.vector.tensor_mul(w[:, :kt, :], g_bc, mf_bc)
                        # x = h_base + sum_q w[q,b] U[q,i], split i-halves
                        pxA = PSA.tile([BC, H1], F32, tag="psA")
                        pxB = PSA.tile([BC, H1], F32, tag="psB")
                        x_stats = ST.tile([BC, 2, 6], F32, tag="stats")
                        npair = kt // 2
                        for hf, ps in ((0, pxA), (1, pxB)):
                            sl = slice(hf * H1, (hf + 1) * H1)
                            nc.tensor.matmul(ps, id_sb, hb16[:, sl],
                                             start=True, stop=False)
                            for kp in range(npair):
                                nc.tensor.matmul(
                                    ps, w[:, 2 * kp:2 * kp + 2, :],
                                    Upi[:, 2 * kp:2 * kp + 2, hf, :],
                                    start=False, stop=(kt % 2 == 0
                                                       and kp == npair - 1),
                                    perf_mode=PM_DR)
                            if kt % 2 == 1:
                                nc.tensor.matmul(
                                    ps, w[:, kt - 1, :], Upi[:, kt - 1, hf, :],
                                    start=False, stop=True)
                            if hf == 0:
                                nc.vector.bn_stats(out=x_stats[:, 0, :],
                                                   in_=ps[:, :HALF])
                        ps_r4, rs_sb = ln_relu(pxA, pxB, x_stats,
                                               last=(_s == S_INNER - 1))

                if t < T - 1:
                    # append u_t = rs*h' with g(tau) folded into Ujb
                    q0 = BC * t
                    k0, p0 = q0 // 128, q0 % 128
                    nc.vector.tensor_scalar(
                        out=Ujb[:, :, q0:q0 + BC], in0=hT,
                        scalar1=ps_r4[:, 0:1], scalar2=ETA * LAM ** (-1 - t),
                        op0=AL.mult, op1=AL.mult)
                    h2v = bass.AP(
                        tensor=h_sb.tensor, offset=h_sb.offset,
                        ap=[h_sb.ap[0], [HALF, 2], [1, HALF]])
                    nc.gpsimd.tensor_scalar(
                        out=Upi[p0:p0 + BC, k0, :, 0:HALF], in0=h2v,
                        scalar1=rs_sb, scalar2=4.0, op0=AL.mult,
                        op1=AL.mult)
                    # u1 aug: column sums of h' via PE, then rs/8 scale
                    for jc in range(4):
                        nc.tensor.matmul(
                            ps_r4[0:BC, 1 + jc // 2:2 + jc // 2],
                            hT[:, jc, :], ones_sb, start=(jc % 2 == 0),
                            stop=(jc % 2 == 1))
                    u1 = ST.tile([BC, 2], F32, tag="u1")
                    nc.vector.tensor_scalar(
                        out=u1, in0=ps_r4[0:BC, 1:3], scalar1=rs_sb,
                        scalar2=0.125, op0=AL.mult, op1=AL.mult)
                    nc.gpsimd.tensor_copy(
                        Upi[p0:p0 + BC, k0, :, HALF], u1)

            # ---- head + loss partials (uses true h = hTs) ----
            ps_p = PSA.tile([BC, D_OUT], F32, tag="psA")
            for jc in range(4):
                nc.tensor.matmul(ps_p, hTs[:, jc, :], HWT_sb[:, jc, :],
                                 start=(jc == 0), stop=(jc == 3))
            pred = W.tile([BC, D_OUT], F32, tag="pred")
            nc.vector.tensor_add(pred, hbias_sb, ps_p)

            def normalize(v_sb, out_sb):
                scr = W.tile([BC, D_OUT], F32, tag="nsq")
                ss = ST.tile([BC, 1], F32, tag="ss")
                rr = ST.tile([BC, 1], F32, tag="rr")
                nc.scalar.activation(scr, v_sb, AF.Square, accum_out=ss)
                nc.scalar.activation(ss, ss, AF.Sqrt)
                nc.vector.tensor_scalar_add(ss, ss, 1e-6)
                nc.vector.reciprocal(rr, ss)
                nc.vector.tensor_scalar_mul(out_sb, v_sb, rr)

            tn = W.tile([BC, D_OUT], F32, tag="tn")
            pn = W.tile([BC, D_OUT], F32, tag="pn")
            normalize(cv_sb, tn)
            normalize(pred, pn)
            diff = W.tile([BC, D_OUT], F32, tag="diff")
            nc.vector.tensor_sub(diff, pn, tn)
            dsq = W.tile([BC, D_OUT], F32, tag="dsq")
            dss = ST.tile([BC, 1], F32, tag="dss")
            nc.scalar.activation(dsq, diff, AF.Square, accum_out=dss)
            h32 = W.tile([BC, D_H], F32, tag="h32")
            nc.vector.tensor_scalar(
                out=h32, in0=h_sb, scalar1=ps_r4[0:BC, 0:1], scalar2=None,
                op0=AL.mult)
            nc.sync.dma_start(out=partial[:], in_=dss[:, 0])
            nc.sync.dma_start(out=h_final[:], in_=h32[:])

    nc.compile()
    return nc


_NC_CACHE = {}


def _get_nc(fast_ln=True):
    if fast_ln not in _NC_CACHE:
        _NC_CACHE[fast_ln] = _build_nc(fast_ln)
    return _NC_CACHE[fast_ln]


def _make_in_maps(inputs):
    return _prep_in_maps(**inputs)[1]


def _prep_in_maps(z_seq, clean_vec, W_h, W_g, b_h, ln_gamma, ln_beta, head_W,
                  head_b):
    z_seq = np.ascontiguousarray(np.asarray(z_seq, np.float32))
    clean_vec = np.ascontiguousarray(np.asarray(clean_vec, np.float32))
    W_h = np.asarray(W_h, np.float32)
    W_g = np.asarray(W_g, np.float32)
    b_h = np.asarray(b_h, np.float32)
    ln_gamma = np.asarray(ln_gamma, np.float32)
    ln_beta = np.asarray(ln_beta, np.float32)
    head_W = np.asarray(head_W, np.float32)
    head_b = np.asarray(head_b, np.float32)

    fast_ln = bool(np.all(ln_gamma == 1.0) and np.all(ln_beta == 0.0))

    def chunk_w(wt, nck, dt=BF16):  # [J, I] -> [128, nck, I]
        J, I = wt.shape
        return np.ascontiguousarray(
            wt.reshape(nck, 128, I).transpose(1, 0, 2).astype(dt))

    def aug_halves(w3):  # [128, n, 512] -> [128, n, 2*257] w/ half-sums/32
        n = w3.shape[1]
        out = np.zeros((128, n, 2, HALF + 1), np.float32)
        for hf in range(2):
            hslice = w3[:, :, hf * HALF:(hf + 1) * HALF]
            out[:, :, hf, :HALF] = hslice
            out[:, :, hf, HALF] = hslice.sum(axis=2) / 32.0
        return np.ascontiguousarray(
            out.reshape(128, n, 2 * (HALF + 1)).astype(BF16))

    WhT = aug_halves(
        W_h.T.reshape(4, 128, D_H).transpose(1, 0, 2).astype(np.float32))
    HWT = chunk_w(head_W.T, 4)
    WgT2 = W_g.T.reshape(2, 128, D_H).transpose(1, 0, 2)
    WgTa0 = np.zeros((128, 3, D_H), np.float32)
    WgTa0[:, :2, :] = WgT2
    WgTa0[0, 2, :] = b_h
    WgTa = aug_halves(WgTa0)

    sel4 = np.zeros((128, 4, BC), np.float32)
    p = np.arange(128)
    for v in range(4):
        for b in range(BC):
            sel4[v * BC + b, v, b] = 1.0
    sel4 = sel4.astype(BF16)
    id32 = np.eye(BC, dtype=BF16)
    mask16 = (p[:, None] % BC == np.arange(BC)[None, :]).astype(BF16)
    maskT = np.ascontiguousarray(mask16.T.astype(np.float32))
    hbias_r = np.ascontiguousarray(
        np.tile(head_b[None, :], (BC, 1)).astype(np.float32))
    gam_r = np.ascontiguousarray(
        np.tile(ln_gamma[None, :], (BC, 1)).astype(np.float32))
    bet_r = np.ascontiguousarray(
        np.tile(ln_beta[None, :], (BC, 1)).astype(np.float32))

    in_maps = []
    for m in range(N_CORES):
        sl = slice(m * BC, (m + 1) * BC)
        zc = z_seq[:, sl, :].transpose(2, 0, 1).reshape(2, 128, NQ)
        zTa = np.zeros((128, 3, NQ), np.float32)
        zTa[:, :2, :] = zc.transpose(1, 0, 2)
        zTa[0, 2, :] = 1.0
        d = {
            "zTa": np.ascontiguousarray(zTa.astype(BF16)),
            "WgTa": WgTa, "WhT": WhT, "HWT": HWT,
            "sel4": sel4, "id32": id32, "mask16": mask16, "maskT": maskT,
            "cleanv": np.ascontiguousarray(clean_vec[sl]),
            "hbias_r": hbias_r,
        }
        if not fast_ln:
            d["gam_r"] = gam_r
            d["bet_r"] = bet_r
        in_maps.append(d)

    return fast_ln, in_maps


def kernel(**inputs):
    fast_ln, in_maps = _prep_in_maps(**inputs)
    nc = _get_nc(fast_ln)
    res = run_bass_kernel_spmd(nc, in_maps, list(range(N_CORES)))
    total = np.float64(0.0)
    for m in range(N_CORES):
        total += np.float64(res.results[m]["partial"].sum())
    loss = total / (B * D_OUT)
    return np.array(loss, dtype=np.float32)


if __name__ == "__main__":
    import reference as ref
    inputs = {k: np.asarray(v) for k, v in ref.setup_inputs().items()}
    out = kernel(**inputs)
    print("kernel loss:", out)
